# revision 24
# baseline (speedup 1.0000x reference)
"""CRF log-likelihood loss kernel for Trainium2 (8 NeuronCores, Bass/Tile).

Strategy (data-parallel over batch, per sharding hint):
  - B=256 batch rows sharded 32 per core; W/b/CRF tables replicated.
  - Host pre-transposes each emissions shard to [H, T, Bs], casts to BF16,
    and splits the time axis into an ascending half (t=0..255) and a
    DESCENDING half (t=511..256).  The device matmul then produces X tiles
    of shape [64, 512] where partitions 0-31 hold x(t) for the forward
    chain and partitions 32-63 hold x(511-t) for the backward chain at the
    SAME column slice.
  - NO per-chain renormalization: the projection bias is shifted by a
    constant c ~= E[log colsum(x)] = 3.95 on device (x_t <- x_t e^-c), so
    partial chain products stay within e^(+-30) for all 255 rounds (bf16
    range is e^(+-88)); the host adds 512c back to every logZ_b.
  - Chain: ONE block-diagonal matmul lhsT = diag(E, E^T) [64,64] advances
    the forward alpha AND backward beta states together, then ONE DVE
    multiply by the fused X slice.  Two batch-half streams (16 cols each)
    interleave to hide the PE->DVE->PE round-trip latency.
  - Z_b = alpha_255^T E (x_256*beta_256) finishes on host in f64, plus the
    gold-path score terms (host einsum over the f32 emissions shard).
"""

import numpy as np

B, T, H, K = 256, 512, 256, 32
NCORES = 8
BS = B // NCORES          # 32 batch rows per core
BH = BS // 2              # 16 batch cols per chain stream
NSTREAM = 2
NT2 = (T // 2) * BS       # 8192 cols per (h, dir) quarter
SUB = 512                 # cols per matmul / X tile
NTILE = NT2 // SUB        # 16 fused X tiles [64, 512]
CHUNK = 1024              # cols per streamed DMA chunk
NCHUNK = NT2 // CHUNK     # 4 chunks per (h, dir)
TS_PER_XT = SUB // BS     # 16 t-steps per X tile
NROUND = 255              # fwd t=1..255 and bwd t=510..256, fused
CSHIFT = 3.95             # per-step log-growth removed from the bias

_BUILT = {}
LAST_RESULTS = None


def _build_nc():
    import concourse.bacc as bacc
    import concourse.tile as tile
    from concourse import mybir
    from contextlib import ExitStack

    f32 = mybir.dt.float32
    bf16 = mybir.dt.bfloat16
    Exp = mybir.ActivationFunctionType.Exp
    Copy = mybir.ActivationFunctionType.Copy
    mult = mybir.AluOpType.mult

    nc = bacc.Bacc("TRN2", target_bir_lowering=False, debug=False,
                   num_devices=NCORES)

    # emisT[h, d]: h = 128-row half of H, d=0 t ascending 0..255,
    # d=1 t descending 511..256; columns are (t, b) b-fastest.
    emisT = nc.declare_dram_parameter("emisT", [2, 2, 128, NT2], bf16,
                                      isOutput=False)
    wpk = nc.declare_dram_parameter("wpk", [128, 2 * K], bf16, isOutput=False)
    eblk = nc.declare_dram_parameter("eblk", [2 * K, 2 * K], bf16,
                                     isOutput=False)
    # cvec: col0 = [b - c; b - c], col1 = [exp(start); exp(end)]
    cvec = nc.declare_dram_parameter("cvec", [2 * K, 2], f32, isOutput=False)
    amv_d = nc.declare_dram_parameter("amv", [2 * K, BS], f32, isOutput=True)

    with ExitStack() as ctx:
        tc = ctx.enter_context(tile.TileContext(nc))
        consts = ctx.enter_context(tc.tile_pool(name="consts", bufs=1))
        emis_pool = ctx.enter_context(tc.tile_pool(name="emis", bufs=1))
        xpool = ctx.enter_context(tc.tile_pool(name="xp", bufs=NTILE))
        apool = ctx.enter_context(tc.tile_pool(name="ap", bufs=3))
        psum_l = ctx.enter_context(tc.tile_pool(name="pl", bufs=4,
                                                space="PSUM"))
        psum_c = ctx.enter_context(tc.tile_pool(name="pc", bufs=2,
                                                space="PSUM"))

        w_sb = consts.tile([128, 2 * K], bf16)
        eblk_sb = consts.tile([2 * K, 2 * K], bf16)
        cvec_sb = consts.tile([2 * K, 2], f32)
        amv_sb = consts.tile([2 * K, BS], f32)
        # early tiles: first SUB of each (h, d) so the chain starts fast
        est = {}
        for h in range(2):
            for d in range(2):
                est[(h, d)] = consts.tile([128, 2 * SUB], bf16,
                                          name=f"est{h}{d}")
        nc.gpsimd.dma_start(out=w_sb, in_=wpk[:, :])
        nc.gpsimd.dma_start(out=cvec_sb, in_=cvec[:, :])
        nc.gpsimd.dma_start(out=eblk_sb, in_=eblk[:, :])
        nc.gpsimd.dma_start(out=est[(0, 1)], in_=emisT[0, 1, :, 0:2 * SUB])
        nc.gpsimd.dma_start(out=est[(1, 1)], in_=emisT[1, 1, :, 0:2 * SUB])
        nc.sync.dma_start(out=est[(0, 0)], in_=emisT[0, 0, :, 0:2 * SUB])
        nc.sync.dma_start(out=est[(1, 0)], in_=emisT[1, 0, :, 0:2 * SUB])
        w0 = w_sb[:, 0:K]
        w1 = w_sb[:, K:2 * K]
        bias64 = cvec_sb[:, 0:1]
        sevec = cvec_sb[:, 1:2]

        # ---- stream emissions chunks: d=0 on sync queue, d=1 on gpsimd ----
        etiles = {}
        for c in range(1, NCHUNK):
            for h in range(2):
                cs, ce = c * CHUNK, (c + 1) * CHUNK
                e0 = emis_pool.tile([128, CHUNK], bf16, tag=f"e{h}0{c}")
                nc.sync.dma_start(out=e0, in_=emisT[h, 0, :, cs:ce])
                etiles[(h, 0, c)] = e0
                e1 = emis_pool.tile([128, CHUNK], bf16, tag=f"e{h}1{c}")
                nc.gpsimd.dma_start(out=e1, in_=emisT[h, 1, :, cs:ce])
                etiles[(h, 1, c)] = e1

        # ---- bulk tile production, interleaved with chain rounds so the
        # chain starts as soon as X tile 0 exists and PE alternates between
        # streaming bulk matmuls and the latency-bound chain matmuls ----
        xtiles = []

        def emit_tile(k):
            pl = psum_l.tile([2 * K, SUB], f32, tag="pl")
            for d in range(2):
                band = pl[d * K:(d + 1) * K, :]
                for h in range(2):
                    if k <= 1:
                        src = est[(h, d)][:, k * SUB:(k + 1) * SUB]
                    else:
                        e = etiles[(h, d, k // (NTILE // NCHUNK))]
                        off = (k % (NTILE // NCHUNK)) * SUB
                        src = e[:, off:off + SUB]
                    nc.tensor.matmul(band, w0 if h == 0 else w1, src,
                                     start=(h == 0), stop=(h == 1))
            xt = xpool.tile([2 * K, SUB], bf16, tag="xt")
            nc.scalar.activation(out=xt, in_=pl, func=Exp, bias=bias64)
            xtiles.append(xt)

        def xslice(r, h):
            k, i = r // TS_PER_XT, r % TS_PER_XT
            c0 = i * BS + h * BH
            return xtiles[k][:, c0:c0 + BH]

        # ---- fused bidirectional chain, no renorm ----
        states = [None] * NSTREAM

        def emit_rounds(lo, hi):
            for r in range(lo, hi):
                for h in range(NSTREAM):
                    pc = psum_c.tile([2 * K, BH], f32, tag=f"pc{h}")
                    nc.tensor.matmul(pc, eblk_sb, states[h],
                                     start=True, stop=True)
                    a_new = apool.tile([2 * K, BH], bf16, tag=f"a{h}")
                    nc.vector.tensor_mul(a_new, pc, xslice(r, h))
                    states[h] = a_new

        LEAD = 3
        emit_tile(0)
        for h in range(NSTREAM):
            st = apool.tile([2 * K, BH], bf16, tag=f"a{h}")
            nc.vector.tensor_scalar(out=st, in0=xslice(0, h),
                                    scalar1=sevec, scalar2=None, op0=mult)
            states[h] = st
        for k in range(1, LEAD):
            emit_tile(k)
        done = 1
        for k in range(LEAD, NTILE):
            emit_tile(k)
            emit_rounds(done, 16 * (k - LEAD + 1))
            done = 16 * (k - LEAD + 1)
        emit_rounds(done, NROUND + 1)

        for h in range(NSTREAM):
            nc.scalar.activation(out=amv_sb[:, h * BH:(h + 1) * BH],
                                 in_=states[h], func=Copy)
        nc.sync.dma_start(out=amv_d[:, :], in_=amv_sb)

    nc.compile()
    return nc


def _numpy_fallback(emissions, W, b, start_transitions, transitions,
                    end_transitions, tags, mask):
    e = emissions.astype(np.float64)
    logits = e @ W.astype(np.float64) + b.astype(np.float64)
    mf = mask.astype(np.float64)
    st = start_transitions.astype(np.float64)
    tr = transitions.astype(np.float64)
    en = end_transitions.astype(np.float64)
    Bn = logits.shape[0]
    bar = np.arange(Bn)
    first = tags[:, 0]
    score = st[first] + logits[bar, 0, first]
    prev = first.copy()
    for t in range(1, T):
        tg = tags[:, t]
        stepv = tr[prev, tg] + logits[bar, t, tg]
        score = score + stepv * mf[:, t]
        prev = np.where(mf[:, t] > 0, tg, prev)
    score = score + en[prev]
    alpha = st[None, :] + logits[:, 0]
    for t in range(1, T):
        nxt = alpha[:, :, None] + tr[None, :, :]
        m = nxt.max(axis=1, keepdims=True)
        nxt = np.log(np.exp(nxt - m).sum(axis=1)) + m[:, 0, :] + logits[:, t]
        alpha = np.where(mf[:, t:t + 1] > 0, nxt, alpha)
    fin = alpha + en[None, :]
    m = fin.max(axis=1, keepdims=True)
    logz = np.log(np.exp(fin - m).sum(axis=1)) + m[:, 0]
    return np.asarray((score - logz).sum(), dtype=np.float32)


def kernel(emissions, W, b, start_transitions, transitions, end_transitions,
           tags, mask):
    global LAST_RESULTS
    emissions = np.ascontiguousarray(np.asarray(emissions, dtype=np.float32))
    W = np.asarray(W, dtype=np.float32)
    b = np.asarray(b, dtype=np.float32)
    start_transitions = np.asarray(start_transitions, dtype=np.float32)
    transitions = np.asarray(transitions, dtype=np.float32)
    end_transitions = np.asarray(end_transitions, dtype=np.float32)
    tags = np.asarray(tags).astype(np.int64)
    mask = np.asarray(mask).astype(bool)

    if not mask.all():
        return _numpy_fallback(emissions, W, b, start_transitions, transitions,
                               end_transitions, tags, mask)

    from concourse.bass_utils import run_bass_kernel_spmd

    if "nc" not in _BUILT:
        _BUILT["nc"] = _build_nc()
    nc = _BUILT["nc"]

    import ml_dtypes
    bf = ml_dtypes.bfloat16
    wpk_h = np.ascontiguousarray(
        W.reshape(2, 128, K).transpose(1, 0, 2).reshape(128, 2 * K).astype(bf))
    E32 = np.exp(transitions).astype(np.float32)
    eblk_h = np.zeros((2 * K, 2 * K), dtype=np.float32)
    eblk_h[:K, :K] = E32          # fwd: lhsT = E
    eblk_h[K:, K:] = E32.T        # bwd: lhsT = E^T
    eblk_h = np.ascontiguousarray(eblk_h.astype(bf))
    cvec_h = np.zeros((2 * K, 2), dtype=np.float32)
    cvec_h[:K, 0] = b - CSHIFT
    cvec_h[K:, 0] = b - CSHIFT
    cvec_h[:K, 1] = np.exp(start_transitions)
    cvec_h[K:, 1] = np.exp(end_transitions)
    cvec_h = np.ascontiguousarray(cvec_h)

    in_maps = []
    emisT_f32 = []
    for c in range(NCORES):
        sh = emissions[c * BS:(c + 1) * BS]                # [BS, T, H]
        shT = np.ascontiguousarray(sh.transpose(2, 1, 0))  # [H, T, BS]
        emisT_f32.append(shT)
        shTb = shT.astype(bf)
        dev = np.empty((2, 2, 128, NT2), dtype=bf)
        for h in range(2):
            blk = shTb[h * 128:(h + 1) * 128]              # [128, T, BS]
            dev[h, 0] = blk[:, :T // 2].reshape(128, NT2)
            dev[h, 1] = blk[:, :T // 2 - 1:-1].reshape(128, NT2)
        in_maps.append(dict(emisT=np.ascontiguousarray(dev), wpk=wpk_h,
                            eblk=eblk_h, cvec=cvec_h))

    res = run_bass_kernel_spmd(nc, in_maps, list(range(NCORES)))
    LAST_RESULTS = res

    E64 = np.exp(transitions.astype(np.float64))
    total = 0.0
    for c in range(NCORES):
        out = res.results[c]
        amv = np.asarray(out["amv"]).astype(np.float64)    # [2K, BS]
        amid = amv[:K, :]                                  # alpha_255
        vmid = amv[K:, :]                                  # x_256*beta_256
        zmid = np.einsum("kb,kj,jb->b", amid, E64, vmid)
        logz = T * CSHIFT + np.log(zmid)
        tg = tags[c * BS:(c + 1) * BS]
        tgflat = tg.T.reshape(-1)                          # t-major, b-fastest
        ef = emisT_f32[c].reshape(H, T * BS)
        gold = np.einsum("hc,hc->", ef, W[:, tgflat], dtype=np.float64)
        hterm = (start_transitions.astype(np.float64)[tg[:, 0]].sum()
                 + transitions.astype(np.float64)[tg[:, :-1], tg[:, 1:]].sum()
                 + end_transitions.astype(np.float64)[tg[:, -1]].sum()
                 + b.astype(np.float64)[tg].sum())
        total += gold + hterm - logz.sum()

    return np.asarray(total, dtype=np.float32)


# revision 25
# speedup vs baseline: 1.0007x; 1.0007x over previous
"""CRF log-likelihood loss kernel for Trainium2 (8 NeuronCores, Bass/Tile).

Strategy (data-parallel over batch, per sharding hint):
  - B=256 batch rows sharded 32 per core; W/b/CRF tables replicated.
  - Host pre-transposes each emissions shard to [H, T, Bs], casts to BF16,
    and splits the time axis into an ascending half (t=0..255) and a
    DESCENDING half (t=511..256).  The device matmul then produces X tiles
    of shape [64, 512] where partitions 0-31 hold x(t) for the forward
    chain and partitions 32-63 hold x(511-t) for the backward chain at the
    SAME column slice.
  - NO per-chain renormalization: the projection bias is shifted by a
    constant c ~= E[log colsum(x)] = 3.95 on device (x_t <- x_t e^-c), so
    partial chain products stay within e^(+-30) for all 255 rounds (bf16
    range is e^(+-88)); the host adds 512c back to every logZ_b.
  - Chain: ONE block-diagonal matmul lhsT = diag(E, E^T) [64,64] advances
    the forward alpha AND backward beta states together, then ONE DVE
    multiply by the fused X slice.  Two batch-half streams (16 cols each)
    interleave to hide the PE->DVE->PE round-trip latency.
  - Z_b = alpha_255^T E (x_256*beta_256) finishes on host in f64, plus the
    gold-path score terms (host einsum over the f32 emissions shard).
"""

import numpy as np

B, T, H, K = 256, 512, 256, 32
NCORES = 8
BS = B // NCORES          # 32 batch rows per core
BH = BS // 2              # 16 batch cols per chain stream
NSTREAM = 2
NT2 = (T // 2) * BS       # 8192 cols per (h, dir) quarter
SUB = 512                 # cols per matmul / X tile
NTILE = NT2 // SUB        # 16 fused X tiles [64, 512]
CHUNK = 1024              # cols per streamed DMA chunk
NCHUNK = NT2 // CHUNK     # 4 chunks per (h, dir)
TS_PER_XT = SUB // BS     # 16 t-steps per X tile
NROUND = 255              # fwd t=1..255 and bwd t=510..256, fused
CSHIFT = 3.95             # per-step log-growth removed from the bias

_BUILT = {}
LAST_RESULTS = None


def _build_nc():
    import concourse.bacc as bacc
    import concourse.tile as tile
    from concourse import mybir
    from contextlib import ExitStack

    f32 = mybir.dt.float32
    bf16 = mybir.dt.bfloat16
    Exp = mybir.ActivationFunctionType.Exp
    Copy = mybir.ActivationFunctionType.Copy
    mult = mybir.AluOpType.mult

    nc = bacc.Bacc("TRN2", target_bir_lowering=False, debug=False,
                   num_devices=NCORES)

    # emisT[h, d]: h = 128-row half of H, d=0 t ascending 0..255,
    # d=1 t descending 511..256; columns are (t, b) b-fastest.
    emisT = nc.declare_dram_parameter("emisT", [2, 2, 128, NT2], bf16,
                                      isOutput=False)
    wpk = nc.declare_dram_parameter("wpk", [128, 2 * K], bf16, isOutput=False)
    eblk = nc.declare_dram_parameter("eblk", [2 * K, 2 * K], bf16,
                                     isOutput=False)
    # cvec: col0 = [b - c; b - c], col1 = [exp(start); exp(end)]
    cvec = nc.declare_dram_parameter("cvec", [2 * K, 2], f32, isOutput=False)
    amv_d = nc.declare_dram_parameter("amv", [2 * K, BS], f32, isOutput=True)

    with ExitStack() as ctx:
        tc = ctx.enter_context(tile.TileContext(nc))
        consts = ctx.enter_context(tc.tile_pool(name="consts", bufs=1))
        emis_pool = ctx.enter_context(tc.tile_pool(name="emis", bufs=1))
        xpool = ctx.enter_context(tc.tile_pool(name="xp", bufs=NTILE))
        apool = ctx.enter_context(tc.tile_pool(name="ap", bufs=3))
        psum_l = ctx.enter_context(tc.tile_pool(name="pl", bufs=4,
                                                space="PSUM"))
        psum_c = ctx.enter_context(tc.tile_pool(name="pc", bufs=2,
                                                space="PSUM"))

        w_sb = consts.tile([128, 2 * K], bf16)
        eblk_sb = consts.tile([2 * K, 2 * K], bf16)
        cvec_sb = consts.tile([2 * K, 2], f32)
        amv_sb = consts.tile([2 * K, BS], f32)
        # early tiles: first SUB of each (h, d) so the chain starts fast
        est = {}
        for h in range(2):
            for d in range(2):
                est[(h, d)] = consts.tile([128, 2 * SUB], bf16,
                                          name=f"est{h}{d}")
        nc.gpsimd.dma_start(out=w_sb, in_=wpk[:, :])
        nc.gpsimd.dma_start(out=cvec_sb, in_=cvec[:, :])
        nc.gpsimd.dma_start(out=eblk_sb, in_=eblk[:, :])
        nc.gpsimd.dma_start(out=est[(0, 1)], in_=emisT[0, 1, :, 0:2 * SUB])
        nc.gpsimd.dma_start(out=est[(1, 1)], in_=emisT[1, 1, :, 0:2 * SUB])
        nc.sync.dma_start(out=est[(0, 0)], in_=emisT[0, 0, :, 0:2 * SUB])
        nc.sync.dma_start(out=est[(1, 0)], in_=emisT[1, 0, :, 0:2 * SUB])
        w0 = w_sb[:, 0:K]
        w1 = w_sb[:, K:2 * K]
        bias64 = cvec_sb[:, 0:1]
        sevec = cvec_sb[:, 1:2]

        # ---- stream emissions chunks: d=0 on sync queue, d=1 on gpsimd ----
        etiles = {}
        for c in range(1, NCHUNK):
            for h in range(2):
                cs, ce = c * CHUNK, (c + 1) * CHUNK
                e0 = emis_pool.tile([128, CHUNK], bf16, tag=f"e{h}0{c}")
                nc.sync.dma_start(out=e0, in_=emisT[h, 0, :, cs:ce])
                etiles[(h, 0, c)] = e0
                e1 = emis_pool.tile([128, CHUNK], bf16, tag=f"e{h}1{c}")
                nc.gpsimd.dma_start(out=e1, in_=emisT[h, 1, :, cs:ce])
                etiles[(h, 1, c)] = e1

        # ---- bulk tile production, interleaved with chain rounds so the
        # chain starts as soon as X tile 0 exists and PE alternates between
        # streaming bulk matmuls and the latency-bound chain matmuls ----
        xtiles = []

        def emit_tile(k):
            pl = psum_l.tile([2 * K, SUB], f32, tag="pl")
            for d in range(2):
                band = pl[d * K:(d + 1) * K, :]
                for h in range(2):
                    if k <= 1:
                        src = est[(h, d)][:, k * SUB:(k + 1) * SUB]
                    else:
                        e = etiles[(h, d, k // (NTILE // NCHUNK))]
                        off = (k % (NTILE // NCHUNK)) * SUB
                        src = e[:, off:off + SUB]
                    nc.tensor.matmul(band, w0 if h == 0 else w1, src,
                                     start=(h == 0), stop=(h == 1))
            xt = xpool.tile([2 * K, SUB], bf16, tag="xt")
            nc.scalar.activation(out=xt, in_=pl, func=Exp, bias=bias64)
            xtiles.append(xt)

        def xslice(r, h):
            k, i = r // TS_PER_XT, r % TS_PER_XT
            c0 = i * BS + h * BH
            return xtiles[k][:, c0:c0 + BH]

        # ---- fused bidirectional chain, no renorm ----
        states = [None] * NSTREAM

        def emit_rounds(lo, hi):
            for r in range(lo, hi):
                for h in range(NSTREAM):
                    pc = psum_c.tile([2 * K, BH], f32, tag=f"pc{h}")
                    nc.tensor.matmul(pc, eblk_sb, states[h],
                                     start=True, stop=True)
                    a_new = apool.tile([2 * K, BH], bf16, tag=f"a{h}")
                    nc.vector.tensor_mul(a_new, pc, xslice(r, h))
                    states[h] = a_new

        LEAD = 5
        emit_tile(0)
        for h in range(NSTREAM):
            st = apool.tile([2 * K, BH], bf16, tag=f"a{h}")
            nc.vector.tensor_scalar(out=st, in0=xslice(0, h),
                                    scalar1=sevec, scalar2=None, op0=mult)
            states[h] = st
        for k in range(1, LEAD):
            emit_tile(k)
        done = 1
        for k in range(LEAD, NTILE):
            emit_tile(k)
            emit_rounds(done, 16 * (k - LEAD + 1))
            done = 16 * (k - LEAD + 1)
        emit_rounds(done, NROUND + 1)

        for h in range(NSTREAM):
            nc.scalar.activation(out=amv_sb[:, h * BH:(h + 1) * BH],
                                 in_=states[h], func=Copy)
        nc.sync.dma_start(out=amv_d[:, :], in_=amv_sb)

    nc.compile()
    return nc


def _numpy_fallback(emissions, W, b, start_transitions, transitions,
                    end_transitions, tags, mask):
    e = emissions.astype(np.float64)
    logits = e @ W.astype(np.float64) + b.astype(np.float64)
    mf = mask.astype(np.float64)
    st = start_transitions.astype(np.float64)
    tr = transitions.astype(np.float64)
    en = end_transitions.astype(np.float64)
    Bn = logits.shape[0]
    bar = np.arange(Bn)
    first = tags[:, 0]
    score = st[first] + logits[bar, 0, first]
    prev = first.copy()
    for t in range(1, T):
        tg = tags[:, t]
        stepv = tr[prev, tg] + logits[bar, t, tg]
        score = score + stepv * mf[:, t]
        prev = np.where(mf[:, t] > 0, tg, prev)
    score = score + en[prev]
    alpha = st[None, :] + logits[:, 0]
    for t in range(1, T):
        nxt = alpha[:, :, None] + tr[None, :, :]
        m = nxt.max(axis=1, keepdims=True)
        nxt = np.log(np.exp(nxt - m).sum(axis=1)) + m[:, 0, :] + logits[:, t]
        alpha = np.where(mf[:, t:t + 1] > 0, nxt, alpha)
    fin = alpha + en[None, :]
    m = fin.max(axis=1, keepdims=True)
    logz = np.log(np.exp(fin - m).sum(axis=1)) + m[:, 0]
    return np.asarray((score - logz).sum(), dtype=np.float32)


def kernel(emissions, W, b, start_transitions, transitions, end_transitions,
           tags, mask):
    global LAST_RESULTS
    emissions = np.ascontiguousarray(np.asarray(emissions, dtype=np.float32))
    W = np.asarray(W, dtype=np.float32)
    b = np.asarray(b, dtype=np.float32)
    start_transitions = np.asarray(start_transitions, dtype=np.float32)
    transitions = np.asarray(transitions, dtype=np.float32)
    end_transitions = np.asarray(end_transitions, dtype=np.float32)
    tags = np.asarray(tags).astype(np.int64)
    mask = np.asarray(mask).astype(bool)

    if not mask.all():
        return _numpy_fallback(emissions, W, b, start_transitions, transitions,
                               end_transitions, tags, mask)

    from concourse.bass_utils import run_bass_kernel_spmd

    if "nc" not in _BUILT:
        _BUILT["nc"] = _build_nc()
    nc = _BUILT["nc"]

    import ml_dtypes
    bf = ml_dtypes.bfloat16
    wpk_h = np.ascontiguousarray(
        W.reshape(2, 128, K).transpose(1, 0, 2).reshape(128, 2 * K).astype(bf))
    E32 = np.exp(transitions).astype(np.float32)
    eblk_h = np.zeros((2 * K, 2 * K), dtype=np.float32)
    eblk_h[:K, :K] = E32          # fwd: lhsT = E
    eblk_h[K:, K:] = E32.T        # bwd: lhsT = E^T
    eblk_h = np.ascontiguousarray(eblk_h.astype(bf))
    cvec_h = np.zeros((2 * K, 2), dtype=np.float32)
    cvec_h[:K, 0] = b - CSHIFT
    cvec_h[K:, 0] = b - CSHIFT
    cvec_h[:K, 1] = np.exp(start_transitions)
    cvec_h[K:, 1] = np.exp(end_transitions)
    cvec_h = np.ascontiguousarray(cvec_h)

    in_maps = []
    emisT_f32 = []
    for c in range(NCORES):
        sh = emissions[c * BS:(c + 1) * BS]                # [BS, T, H]
        shT = np.ascontiguousarray(sh.transpose(2, 1, 0))  # [H, T, BS]
        emisT_f32.append(shT)
        shTb = shT.astype(bf)
        dev = np.empty((2, 2, 128, NT2), dtype=bf)
        for h in range(2):
            blk = shTb[h * 128:(h + 1) * 128]              # [128, T, BS]
            dev[h, 0] = blk[:, :T // 2].reshape(128, NT2)
            dev[h, 1] = blk[:, :T // 2 - 1:-1].reshape(128, NT2)
        in_maps.append(dict(emisT=np.ascontiguousarray(dev), wpk=wpk_h,
                            eblk=eblk_h, cvec=cvec_h))

    res = run_bass_kernel_spmd(nc, in_maps, list(range(NCORES)))
    LAST_RESULTS = res

    E64 = np.exp(transitions.astype(np.float64))
    total = 0.0
    for c in range(NCORES):
        out = res.results[c]
        amv = np.asarray(out["amv"]).astype(np.float64)    # [2K, BS]
        amid = amv[:K, :]                                  # alpha_255
        vmid = amv[K:, :]                                  # x_256*beta_256
        zmid = np.einsum("kb,kj,jb->b", amid, E64, vmid)
        logz = T * CSHIFT + np.log(zmid)
        tg = tags[c * BS:(c + 1) * BS]
        tgflat = tg.T.reshape(-1)                          # t-major, b-fastest
        ef = emisT_f32[c].reshape(H, T * BS)
        gold = np.einsum("hc,hc->", ef, W[:, tgflat], dtype=np.float64)
        hterm = (start_transitions.astype(np.float64)[tg[:, 0]].sum()
                 + transitions.astype(np.float64)[tg[:, :-1], tg[:, 1:]].sum()
                 + end_transitions.astype(np.float64)[tg[:, -1]].sum()
                 + b.astype(np.float64)[tg].sum())
        total += gold + hterm - logz.sum()

    return np.asarray(total, dtype=np.float32)
